# revision 17
# baseline (speedup 1.0000x reference)
"""CRF negative-log-likelihood loss on 8 Trainium2 NeuronCores.

Problem: nn_CRF (B=64, L=8192, T=48), data-parallel over batch (8 rows/core).

Algorithm (device side): the CRF forward recursion in probability space is
    a_l = (a_{l-1} @ E) * d_l,   E = exp(transitions), d_l = exp(e_l - kappa)
Column scaling commutes through the matmul, so with the state TRANSPOSED
([tags, ncols]) every step is ONE matmul by a fixed block-diagonal 96x96
matrix diag(E, E) plus one elementwise multiply. The sequence is split into
CPB=1024 chunks x CLEN=8 steps per batch row; all 4096 chunks of a core run
as independent columns of a [96, 4096] recursion (2 stacked groups of 48
tags, batch row j = column block j*1024). Each chunk starts W=1 steps early
from a uniform vector u; the Birkhoff contraction of E makes the chunk-start
direction error negligible. Chunk 0 of each row is recomputed exactly on the
host in float64.

The warmup step costs the device nothing: E^T u = rho is a fixed per-tag
vector, so X_1 = rho * d(l0-1). The host uint8-log-codes X_1 directly into
a single fused "boot" tensor (weight bytes + activation biases + X_1
codes, one DMA receipt for all of startup) and ScalarE rebuilds it with
one fused exp per wave. Only CLEN emission slabs ship, zero duplication,
and the checkpoint column sums are reproduced host-side instead of DMA'd
out. Startup matters: every DMA trigger stays on the sync queue, because
any gpsimd instruction (including the framework's own const-AP memsets,
stripped in a post-pass below) adds a ~6us Q7 IRAM load to the kernel's
entry barrier.

Engine balance per step (the binding resources are HBM at ~190 GB/s/core
effective and DVE): wave 0 (batch row j=0) ships uint8 log-codes which
ScalarE exp-decodes (halving its HBM bytes) and DVE multiplies from PSUM
at 1x; waves 1-2 ship bf16, ScalarE evacuates their PSUM f32 -> SBUF bf16
and DVE multiplies packed bf16 at 2x_1P; wave 3 ships bf16 and DVE
multiplies it straight from PSUM at 1x. ACT-dependent waves run first in
each step so their two-hop chains hide behind the remaining matmuls.

The host telescopes per-chunk log-mass ratios into log Z in float64 and
subtracts the (host-computed) gold path score. Validated on HW: max rel
err ~2.6e-5; 46.8us on-device vs the 59.5us session baseline.
"""

import numpy as np
import ml_dtypes

bf16 = ml_dtypes.bfloat16

# ---- problem constants (hardcoded per contract) ----
B, L, T = 64, 8192, 48
NCORES = 8
B_CORE = B // NCORES      # 8 batch rows per core
G = 2                     # stacked groups (partitions 0:48 and 48:96)
GP = G * T                # 96 partitions in use
JB = 4                    # batch rows per group
CLEN = 8                  # steps per chunk = device slabs
CPB = L // CLEN           # 1024 chunks per batch row
R = CPB * JB              # 4096 recursion columns per group
W = 1                     # warmup steps (free: folded into X_1)
KAPPA = 4.356             # per-step log-mass shift (E[logZ]/L for this data)
CW = 1024                 # columns per wave == CPB, so wave w = batch row j=w
QLO, QHI = -5.5, 5.5      # uint8 code range for wave-0 emissions
QSCALE = (QHI - QLO) / 255.0
BXLO, BXHI = -10.0, 1.3   # uint8 code range for log X_1 (the boot state)
QSX = (BXHI - BXLO) / 255.0
BOOT_W = 200              # boot tensor: wmat bytes [0:192), ebias f32
                          # [192:196), biasx f32 [196:200), then X_1 codes
XPOS = {1: 0, 2: 1, 0: 2, 3: 3}  # X_1 code layout, matmul-issue order
NB8 = CLEN * CW           # uint8 slab-space columns
NBB = CLEN * 3 * CW       # bf16 slab-space columns (waves 1-3)

_CACHE = {}


def _build_nc():
    import concourse.bacc as bacc
    import concourse.tile as tile
    from concourse import mybir

    nc = bacc.Bacc("TRN2", debug=False)
    boot = nc.dram_tensor("boot", [GP, BOOT_W + R], mybir.dt.uint8, kind="ExternalInput")
    dq8 = nc.dram_tensor("dq8", [GP, NB8], mybir.dt.uint8, kind="ExternalInput")
    dqb = nc.dram_tensor("dqb", [GP, NBB], mybir.dt.bfloat16, kind="ExternalInput")
    xfin = nc.dram_tensor("xfin", [GP, R], mybir.dt.bfloat16, kind="ExternalOutput")

    with tile.TileContext(nc) as tc:
        from contextlib import ExitStack

        with ExitStack() as ctx:
            pool = ctx.enter_context(tc.tile_pool(name="persist", bufs=1))
            psum_pool = ctx.enter_context(
                tc.tile_pool(name="psum", bufs=1, space="PSUM")
            )

            Boot = pool.tile([GP, BOOT_W + R], mybir.dt.uint8)
            Dq8 = pool.tile([GP, NB8], mybir.dt.uint8)
            Dqb = pool.tile([GP, NBB], mybir.dt.bfloat16)
            Dt8 = pool.tile([GP, NB8], mybir.dt.bfloat16)
            Wt = Boot[:, 0:192].bitcast(mybir.dt.bfloat16)
            ebias = Boot[:, 192:196].bitcast(mybir.dt.float32)
            biasx = Boot[:, 196:200].bitcast(mybir.dt.float32)
            Xq8 = Boot[:, BOOT_W:]

            # DMA ingest: every trigger on the sync queue (the gpsimd queue
            # would pull in the Q7 engine, whose ~6us IRAM load holds up the
            # kernel-entry all-engine barrier). One fused boot chunk carries
            # the weights, activation biases, and the uint8-coded X_1, so a
            # single completion receipt gates all of startup; then slabs
            # ship in consumption order.
            bsplit = BOOT_W + 2 * CW
            nc.sync.dma_start(out=Boot[:, 0:bsplit], in_=boot[:, 0:bsplit])
            nc.sync.dma_start(out=Boot[:, bsplit:], in_=boot[:, bsplit:])
            nc.sync.dma_start(out=Dq8[:, 0:CW], in_=dq8[:, 0:CW])
            for p in range(CLEN):
                lo = p * 3 * CW
                nc.sync.dma_start(out=Dqb[:, lo : lo + 3 * CW], in_=dqb[:, lo : lo + 3 * CW])
                if p == 0:
                    nc.sync.dma_start(out=Dq8[:, CW : 4 * CW], in_=dq8[:, CW : 4 * CW])
                elif p == 3:
                    nc.sync.dma_start(out=Dq8[:, 4 * CW :], in_=dq8[:, 4 * CW :])

            X0 = pool.tile([GP, R], mybir.dt.bfloat16)
            X1 = pool.tile([GP, R], mybir.dt.bfloat16)
            X2 = pool.tile([GP, R], mybir.dt.bfloat16)
            X3 = pool.tile([GP, R], mybir.dt.bfloat16)
            Xs = [X0, X1, X2, X3]
            Xinit = pool.tile([GP, R], mybir.dt.bfloat16)

            ps = []
            Pw = []
            for w in range(4):
                pw_ps = psum_pool.tile([GP, CW], mybir.dt.float32, tag=f"psum{w}")
                ps.append(pw_ps)
            for w in (1, 2):
                for par in (0, 1):
                    pw_sb = pool.tile([GP, CW], mybir.dt.bfloat16,
                                      tag=f"pcopy{w}_{par}")
                    Pw.append(pw_sb)

            # touch the Exp table with no real deps beyond the boot chunk so
            # the ACT table load happens during DMA startup
            scratch = pool.tile([GP, 1], mybir.dt.bfloat16)
            nc.scalar.activation(
                out=scratch[:], in_=ebias,
                func=mybir.ActivationFunctionType.Exp, bias=ebias,
            )

            # ---- X_1 = exp(QSX * q + BXLO), one ACT decode per wave ----
            # codes are stored in matmul-issue order so wave 1 decodes as
            # soon as the first boot chunk lands
            for w in (1, 2, 0, 3):
                lo = XPOS[w] * CW
                nc.scalar.activation(
                    out=Xinit[:, w * CW : (w + 1) * CW],
                    in_=Xq8[:, lo : lo + CW],
                    func=mybir.ActivationFunctionType.Exp,
                    bias=biasx, scale=QSX,
                )

            # decode slab 0 for step 1's wave-0 multiply
            nc.scalar.activation(
                out=Dt8[:, 0:CW], in_=Dq8[:, 0:CW],
                func=mybir.ActivationFunctionType.Exp,
                bias=ebias, scale=QSCALE,
            )

            for s in range(1, CLEN + 1):
                p = s - 1
                cur = Xinit if s == 1 else Xs[(s - 1) % 4]
                nxt = Xs[s % 4]
                b8 = p * CW
                bb = p * 3 * CW
                # ACT-dependent waves (w1, w2) run first so their 2-hop
                # chains hide behind PE's remaining matmuls; the final PE
                # op's consumer chain stays short. Step 8 flips the order
                # so the kernel's last DVE op is a short 2x multiply.
                for w in (1, 2, 0, 3):
                    for h in (0, 512):
                        nc.tensor.matmul(
                            ps[w][:, h : h + 512], lhsT=Wt[:],
                            rhs=cur[:, w * CW + h : w * CW + h + 512],
                            start=True, stop=True,
                        )
                    if w == 0:
                        nc.vector.tensor_mul(
                            nxt[:, 0:CW], ps[0][:], Dt8[:, b8 : b8 + CW],
                        )
                    elif w == 3:
                        nc.vector.tensor_mul(
                            nxt[:, 3 * CW : 4 * CW], ps[3][:],
                            Dqb[:, bb + 2 * CW : bb + 3 * CW],
                        )
                    else:
                        pwt = Pw[2 * (w - 1) + (s % 2)]
                        nc.scalar.activation(
                            out=pwt[:], in_=ps[w][:],
                            func=mybir.ActivationFunctionType.Copy,
                        )
                        nc.vector.tensor_mul(
                            nxt[:, w * CW : (w + 1) * CW], pwt[:],
                            Dqb[:, bb + (w - 1) * CW : bb + w * CW],
                        )
                    if s == CLEN:
                        cs = slice(w * CW, (w + 1) * CW)
                        nc.sync.dma_start(out=xfin[:, cs], in_=nxt[:, cs])

                # prefetch-decode the next uint8 slab last: it has no
                # consumers this step, so it must not head-of-line-block
                # the copies in the ACT FIFO
                if s < CLEN:
                    nc.scalar.activation(
                        out=Dt8[:, b8 + CW : b8 + 2 * CW],
                        in_=Dq8[:, b8 + CW : b8 + 2 * CW],
                        func=mybir.ActivationFunctionType.Exp,
                        bias=ebias, scale=QSCALE,
                    )



    # Drop the framework's Pool-engine const-AP memsets: they are the only
    # Q7-ucode instructions in the kernel, and their IRAM load (~6us) holds
    # up the kernel-entry all-engine barrier. They carry no sync_info and
    # nothing reads the const APs (every activation passes an AP bias).
    for blk in nc.m.functions[0].blocks:
        keep = [
            ins for ins in blk.instructions
            if not (isinstance(ins, mybir.InstMemset)
                    and ins.engine == mybir.EngineType.Pool
                    and ins.sync_info is None)
        ]
        if len(keep) != len(blk.instructions):
            blk.instructions[:] = keep

    # The stationary operand never changes: keep only the first LDWEIGHTS.
    seen_ldw = False
    for blk in nc.m.functions[0].blocks:
        keep = []
        for ins in blk.instructions:
            if isinstance(ins, mybir.InstLdweights):
                if seen_ldw:
                    si = ins.sync_info
                    if si is not None and si.on_wait:
                        keep.append(ins)
                    continue
                seen_ldw = True
            keep.append(ins)
        if len(keep) != len(blk.instructions):
            blk.instructions[:] = keep

    nc.compile()
    return nc


def _get_nc():
    if "nc" not in _CACHE:
        _CACHE["nc"] = _build_nc()
    return _CACHE["nc"]


def _build_wmat(E_d):
    wmat = np.zeros((GP, GP), dtype=bf16)
    wmat[0:T, 0:T] = E_d
    wmat[T:GP, T:GP] = E_d
    return wmat


def _build_core_inputs(e_core, wmat_bytes, logrho):
    """Build the per-core input map. e_core: [B_CORE, L, T] f32.
    Returns (inputs, sum8) where sum8 mirrors the device's decoded X_1."""
    c_idx = np.arange(CPB)
    p_idx = np.arange(CLEN)
    l_of = c_idx[:, None] * CLEN + p_idx[None, :]  # [CPB, CLEN], l = 8c+p
    lm1 = np.maximum(c_idx * CLEN - 1, 0)          # warmup source, l0-1

    dq8 = np.empty((GP, NB8), dtype=np.uint8)
    dqb = np.empty((GP, NBB), dtype=bf16)
    xq8 = np.empty((GP, R), dtype=np.uint8)
    sum8 = np.zeros((G, JB, CPB))
    for g in range(G):
        rows = slice(g * T, (g + 1) * T)
        b0 = g * JB
        q = np.clip(np.round((e_core[b0] - QLO) / QSCALE), 0, 255).astype(np.uint8)
        # [CPB, CLEN, T] -> [T, CLEN, CPB]
        dq8[rows] = q[l_of, :].transpose(2, 1, 0).reshape(T, NB8)
        vb = dqb[rows].reshape(T, CLEN, 3, CPB)
        for j in (1, 2, 3):
            De = np.exp(e_core[b0 + j].astype(np.float32) - KAPPA).astype(bf16)
            vb[:, :, j - 1, :] = De[l_of, :].transpose(2, 1, 0)
        for j in range(JB):
            b = b0 + j
            val = logrho[None, :] + e_core[b, lm1, :] - KAPPA  # [CPB, T]
            qx = np.clip(np.round((val - BXLO) / QSX), 0, 255).astype(np.uint8)
            xq8[rows, XPOS[j] * CPB : (XPOS[j] + 1) * CPB] = qx.T
            x1 = np.exp(QSX * qx.astype(np.float32) + BXLO).astype(bf16)
            sum8[g, j] = x1.astype(np.float64).sum(1)
    boot = np.empty((GP, BOOT_W + R), dtype=np.uint8)
    boot[:, 0:192] = wmat_bytes
    boot[:, 192:196] = np.frombuffer(
        np.full(GP, QLO - KAPPA, np.float32).tobytes(), np.uint8
    ).reshape(GP, 4)
    boot[:, 196:200] = np.frombuffer(
        np.full(GP, BXLO, np.float32).tobytes(), np.uint8
    ).reshape(GP, 4)
    boot[:, BOOT_W:] = xq8
    return {"boot": boot, "dq8": dq8, "dqb": dqb}, sum8


def _chunk0_logsum(e_b, start_f, Ef64):
    """Exact log sum(alpha_{CLEN-1}) for one batch row, float64."""
    a = np.exp(start_f.astype(np.float64) + e_b[0].astype(np.float64))
    for l in range(1, CLEN):
        m = a.max()
        a = ((a / m) @ Ef64) * np.exp(e_b[l].astype(np.float64))
        a *= m
    return np.log(a.sum())


def _assemble_core(xfin, sum8, e_core, start_f, end_f, Ef64):
    """Host combine for one core -> logZ [B_CORE] (float64)."""
    w = np.exp(end_f.astype(np.float64))
    logZ = np.zeros(B_CORE)
    for g in range(G):
        rows = slice(g * T, (g + 1) * T)
        s72 = xfin[rows].astype(np.float64)
        sum72 = s72.sum(0)
        for j in range(JB):
            b = g * JB + j
            cols = slice(j * CPB, (j + 1) * CPB)
            A = np.log(sum72[cols]) + CLEN * KAPPA
            A[1:] -= np.log(sum8[g, j, 1:])
            A0 = _chunk0_logsum(e_core[b], start_f, Ef64)
            xlast = s72[:, j * CPB + (CPB - 1)]
            logZ[b] = A0 + A[1:].sum() + np.log(xlast @ w) - np.log(xlast.sum())
    return logZ


def _host_score(emissions, tags, mask, transitions, start_f, end_f):
    tags = np.asarray(tags).astype(np.int64)
    maskf = np.asarray(mask).astype(np.float64)
    emit = np.take_along_axis(
        emissions, tags[:, :, None], axis=2
    )[..., 0].astype(np.float64)
    score = start_f.astype(np.float64)[tags[:, 0]] + (emit * maskf).sum(1)
    tr = transitions.astype(np.float64)[tags[:, :-1], tags[:, 1:]]
    score += (tr * maskf[:, 1:]).sum(1)
    last_idx = maskf.astype(np.int64).sum(1) - 1
    last_tags = np.take_along_axis(tags, last_idx[:, None], axis=1)[:, 0]
    score += end_f.astype(np.float64)[last_tags]
    return score


def kernel(
    emissions, tags, mask, transitions, start_transitions, end_transitions,
    _trace=False,
):
    from concourse.bass_utils import run_bass_kernel_spmd

    emissions = np.asarray(emissions, dtype=np.float32)
    transitions = np.asarray(transitions, dtype=np.float32)
    start_f = np.asarray(start_transitions, dtype=np.float32)
    end_f = np.asarray(end_transitions, dtype=np.float32)

    E_d = np.exp(transitions).astype(bf16)
    Ef64 = np.exp(transitions.astype(np.float64))
    wmat = _build_wmat(E_d)
    wmat_bytes = wmat.view(np.uint8)  # [GP, 192]
    # rho = E^T @ uniform: the warmup step's matmul output direction
    logrho = np.log(Ef64.sum(0) / T).astype(np.float32)  # [T]

    in_maps = []
    sum8s = []
    for core in range(NCORES):
        e_core = emissions[core * B_CORE : (core + 1) * B_CORE]
        im, s8 = _build_core_inputs(e_core, wmat_bytes, logrho)
        in_maps.append(im)
        sum8s.append(s8)

    nc = _get_nc()
    res = run_bass_kernel_spmd(
        nc, in_maps, core_ids=list(range(NCORES)), trace=_trace
    )
    _CACHE["last_results"] = res

    logZ = np.zeros(B)
    for core in range(NCORES):
        out = res.results[core]
        e_core = emissions[core * B_CORE : (core + 1) * B_CORE]
        logZ[core * B_CORE : (core + 1) * B_CORE] = _assemble_core(
            out["xfin"], sum8s[core], e_core, start_f, end_f, Ef64
        )

    score = _host_score(
        emissions, tags, mask, transitions, start_f, end_f
    )
    return (logZ - score).astype(np.float32)


# revision 18
# speedup vs baseline: 1.0198x; 1.0198x over previous
"""CRF negative-log-likelihood loss on 8 Trainium2 NeuronCores.

Problem: nn_CRF (B=64, L=8192, T=48), data-parallel over batch (8 rows/core).

Algorithm (device side): the CRF forward recursion in probability space is
    a_l = (a_{l-1} @ E) * d_l,   E = exp(transitions), d_l = exp(e_l - kappa)
Column scaling commutes through the matmul, so with the state TRANSPOSED
([tags, ncols]) every step is ONE matmul by a fixed block-diagonal 96x96
matrix diag(E, E) plus one elementwise multiply. The sequence is split into
CPB=1024 chunks x CLEN=8 steps per batch row; all 4096 chunks of a core run
as independent columns of a [96, 4096] recursion (2 stacked groups of 48
tags, batch row j = column block j*1024). Each chunk starts W=1 steps early
from a uniform vector u; the Birkhoff contraction of E makes the chunk-start
direction error negligible. Chunk 0 of each row is recomputed exactly on the
host in float64.

The warmup step costs the device nothing: E^T u = rho is a fixed per-tag
vector, so X_1 = rho * d(l0-1). The host uint8-log-codes X_1 directly into
a single fused "boot" tensor (weight bytes + activation biases + X_1
codes, one DMA receipt for all of startup) and ScalarE rebuilds it with
one fused exp per wave. Only CLEN emission slabs ship, zero duplication,
and the checkpoint column sums are reproduced host-side instead of DMA'd
out. Startup matters: every DMA trigger stays on the sync queue, because
any gpsimd instruction (including the framework's own const-AP memsets,
stripped in a post-pass below) adds a ~6us Q7 IRAM load to the kernel's
entry barrier.

Engine balance per step (the binding resources are HBM at ~190 GB/s/core
effective and DVE): wave 0 (batch row j=0) ships uint8 log-codes which
ScalarE exp-decodes (halving its HBM bytes) and DVE multiplies from PSUM
at 1x; waves 1-2 ship bf16, ScalarE evacuates their PSUM f32 -> SBUF bf16
and DVE multiplies packed bf16 at 2x_1P; wave 3 ships bf16 and DVE
multiplies it straight from PSUM at 1x. ACT-dependent waves run first in
each step so their two-hop chains hide behind the remaining matmuls.

The host telescopes per-chunk log-mass ratios into log Z in float64 and
subtracts the (host-computed) gold path score. Validated on HW: max rel
err ~2.6e-5; 46.8us on-device vs the 59.5us session baseline.
"""

import numpy as np
import ml_dtypes

bf16 = ml_dtypes.bfloat16

# ---- problem constants (hardcoded per contract) ----
B, L, T = 64, 8192, 48
NCORES = 8
B_CORE = B // NCORES      # 8 batch rows per core
G = 2                     # stacked groups (partitions 0:48 and 48:96)
GP = G * T                # 96 partitions in use
JB = 4                    # batch rows per group
CLEN = 8                  # steps per chunk = device slabs
CPB = L // CLEN           # 1024 chunks per batch row
R = CPB * JB              # 4096 recursion columns per group
W = 1                     # warmup steps (free: folded into X_1)
KAPPA = 4.356             # per-step log-mass shift (E[logZ]/L for this data)
CW = 1024                 # columns per wave == CPB, so wave w = batch row j=w
QLO, QHI = -5.5, 5.5      # uint8 code range for wave-0 emissions
QSCALE = (QHI - QLO) / 255.0
BXLO, BXHI = -10.0, 1.3   # uint8 code range for log X_1 (the boot state)
QSX = (BXHI - BXLO) / 255.0
BOOT_W = 200              # boot tensor: wmat bytes [0:192), ebias f32
                          # [192:196), biasx f32 [196:200), then X_1 codes
XPOS = {1: 0, 2: 1, 0: 2, 3: 3}  # X_1 code layout, matmul-issue order
NB8 = CLEN * CW           # uint8 slab-space columns
NBB = CLEN * 3 * CW       # bf16 slab-space columns (waves 1-3)

_CACHE = {}


def _build_nc():
    import concourse.bacc as bacc
    import concourse.tile as tile
    from concourse import mybir

    nc = bacc.Bacc("TRN2", debug=False)
    boot = nc.dram_tensor("boot", [GP, BOOT_W + R], mybir.dt.uint8, kind="ExternalInput")
    dq8 = nc.dram_tensor("dq8", [GP, NB8], mybir.dt.uint8, kind="ExternalInput")
    dqb = nc.dram_tensor("dqb", [GP, NBB], mybir.dt.bfloat16, kind="ExternalInput")
    xfin = nc.dram_tensor("xfin", [GP, R], mybir.dt.bfloat16, kind="ExternalOutput")

    with tile.TileContext(nc) as tc:
        from contextlib import ExitStack

        with ExitStack() as ctx:
            pool = ctx.enter_context(tc.tile_pool(name="persist", bufs=1))
            psum_pool = ctx.enter_context(
                tc.tile_pool(name="psum", bufs=1, space="PSUM")
            )

            Boot = pool.tile([GP, BOOT_W + R], mybir.dt.uint8)
            Dq8 = pool.tile([GP, NB8], mybir.dt.uint8)
            Dqb = pool.tile([GP, NBB], mybir.dt.bfloat16)
            Dt8 = pool.tile([GP, NB8], mybir.dt.bfloat16)
            Wt = Boot[:, 0:192].bitcast(mybir.dt.bfloat16)
            ebias = Boot[:, 192:196].bitcast(mybir.dt.float32)
            biasx = Boot[:, 196:200].bitcast(mybir.dt.float32)
            Xq8 = Boot[:, BOOT_W:]

            # DMA ingest: every trigger on the sync queue (the gpsimd queue
            # would pull in the Q7 engine, whose ~6us IRAM load holds up the
            # kernel-entry all-engine barrier). One fused boot chunk carries
            # the weights, activation biases, and the uint8-coded X_1, so a
            # single completion receipt gates all of startup; then slabs
            # ship in consumption order.
            bsplit = BOOT_W + 2 * CW
            nc.sync.dma_start(out=Boot[:, 0:bsplit], in_=boot[:, 0:bsplit])
            nc.sync.dma_start(out=Boot[:, bsplit:], in_=boot[:, bsplit:])
            nc.sync.dma_start(out=Dq8[:, 0:CW], in_=dq8[:, 0:CW])
            for p in range(CLEN):
                lo = p * 3 * CW
                nc.sync.dma_start(out=Dqb[:, lo : lo + 3 * CW], in_=dqb[:, lo : lo + 3 * CW])
                if p == 0:
                    nc.sync.dma_start(out=Dq8[:, CW : 4 * CW], in_=dq8[:, CW : 4 * CW])
                elif p == 3:
                    nc.sync.dma_start(out=Dq8[:, 4 * CW :], in_=dq8[:, 4 * CW :])

            X0 = pool.tile([GP, R], mybir.dt.bfloat16)
            X1 = pool.tile([GP, R], mybir.dt.bfloat16)
            X2 = pool.tile([GP, R], mybir.dt.bfloat16)
            X3 = pool.tile([GP, R], mybir.dt.bfloat16)
            Xs = [X0, X1, X2, X3]
            Xinit = pool.tile([GP, R], mybir.dt.bfloat16)

            ps = []
            Pw = []
            for w in range(4):
                pw_ps = psum_pool.tile([GP, CW], mybir.dt.float32, tag=f"psum{w}")
                ps.append(pw_ps)
            for w in (1, 2):
                for par in (0, 1):
                    pw_sb = pool.tile([GP, CW], mybir.dt.bfloat16,
                                      tag=f"pcopy{w}_{par}")
                    Pw.append(pw_sb)

            # touch the Exp table with no real deps beyond the boot chunk so
            # the ACT table load happens during DMA startup
            scratch = pool.tile([GP, 1], mybir.dt.bfloat16)
            nc.scalar.activation(
                out=scratch[:], in_=ebias,
                func=mybir.ActivationFunctionType.Exp, bias=ebias,
            )

            # ---- X_1 = exp(QSX * q + BXLO), one ACT decode per wave ----
            # codes are stored in matmul-issue order so wave 1 decodes as
            # soon as the first boot chunk lands
            for w in (1, 2, 0, 3):
                lo = XPOS[w] * CW
                nc.scalar.activation(
                    out=Xinit[:, w * CW : (w + 1) * CW],
                    in_=Xq8[:, lo : lo + CW],
                    func=mybir.ActivationFunctionType.Exp,
                    bias=biasx, scale=QSX,
                )

            # decode slab 0 for step 1's wave-0 multiply
            nc.scalar.activation(
                out=Dt8[:, 0:CW], in_=Dq8[:, 0:CW],
                func=mybir.ActivationFunctionType.Exp,
                bias=ebias, scale=QSCALE,
            )

            for s in range(1, CLEN + 1):
                p = s - 1
                cur = Xinit if s == 1 else Xs[(s - 1) % 4]
                nxt = Xs[s % 4]
                b8 = p * CW
                bb = p * 3 * CW
                # ACT-dependent waves (w1, w2) run first so their 2-hop
                # chains hide behind PE's remaining matmuls; the final PE
                # op's consumer chain stays short. Step 8 flips the order
                # so the kernel's last DVE op is a short 2x multiply.
                for w in (1, 2, 0, 3):
                    for h in (0, 512):
                        nc.tensor.matmul(
                            ps[w][:, h : h + 512], lhsT=Wt[:],
                            rhs=cur[:, w * CW + h : w * CW + h + 512],
                            start=True, stop=True,
                        )
                    if w == 0:
                        nc.vector.tensor_mul(
                            nxt[:, 0:CW], ps[0][:], Dt8[:, b8 : b8 + CW],
                        )
                    elif w == 3:
                        nc.vector.tensor_mul(
                            nxt[:, 3 * CW : 4 * CW], ps[3][:],
                            Dqb[:, bb + 2 * CW : bb + 3 * CW],
                        )
                    else:
                        pwt = Pw[2 * (w - 1) + (s % 2)]
                        nc.scalar.activation(
                            out=pwt[:], in_=ps[w][:],
                            func=mybir.ActivationFunctionType.Copy,
                        )
                        nc.vector.tensor_mul(
                            nxt[:, w * CW : (w + 1) * CW], pwt[:],
                            Dqb[:, bb + (w - 1) * CW : bb + w * CW],
                        )


                # prefetch-decode the next uint8 slab last: it has no
                # consumers this step, so it must not head-of-line-block
                # the copies in the ACT FIFO
                if s < CLEN:
                    nc.scalar.activation(
                        out=Dt8[:, b8 + CW : b8 + 2 * CW],
                        in_=Dq8[:, b8 + CW : b8 + 2 * CW],
                        func=mybir.ActivationFunctionType.Exp,
                        bias=ebias, scale=QSCALE,
                    )



            fin = Xs[CLEN % 4]
            for w in (1, 2, 0, 3):
                cs = slice(w * CW, (w + 1) * CW)
                nc.sync.dma_start(out=xfin[:, cs], in_=fin[:, cs])

    # Drop the framework's Pool-engine const-AP memsets: they are the only
    # Q7-ucode instructions in the kernel, and their IRAM load (~6us) holds
    # up the kernel-entry all-engine barrier. They carry no sync_info and
    # nothing reads the const APs (every activation passes an AP bias).
    for blk in nc.m.functions[0].blocks:
        keep = [
            ins for ins in blk.instructions
            if not (isinstance(ins, mybir.InstMemset)
                    and ins.engine == mybir.EngineType.Pool
                    and ins.sync_info is None)
        ]
        if len(keep) != len(blk.instructions):
            blk.instructions[:] = keep

    # The stationary operand never changes: keep only the first LDWEIGHTS.
    seen_ldw = False
    for blk in nc.m.functions[0].blocks:
        keep = []
        for ins in blk.instructions:
            if isinstance(ins, mybir.InstLdweights):
                if seen_ldw:
                    si = ins.sync_info
                    if si is not None and si.on_wait:
                        keep.append(ins)
                    continue
                seen_ldw = True
            keep.append(ins)
        if len(keep) != len(blk.instructions):
            blk.instructions[:] = keep

    nc.compile()
    return nc


def _get_nc():
    if "nc" not in _CACHE:
        _CACHE["nc"] = _build_nc()
    return _CACHE["nc"]


def _build_wmat(E_d):
    wmat = np.zeros((GP, GP), dtype=bf16)
    wmat[0:T, 0:T] = E_d
    wmat[T:GP, T:GP] = E_d
    return wmat


def _build_core_inputs(e_core, wmat_bytes, logrho):
    """Build the per-core input map. e_core: [B_CORE, L, T] f32.
    Returns (inputs, sum8) where sum8 mirrors the device's decoded X_1."""
    c_idx = np.arange(CPB)
    p_idx = np.arange(CLEN)
    l_of = c_idx[:, None] * CLEN + p_idx[None, :]  # [CPB, CLEN], l = 8c+p
    lm1 = np.maximum(c_idx * CLEN - 1, 0)          # warmup source, l0-1

    dq8 = np.empty((GP, NB8), dtype=np.uint8)
    dqb = np.empty((GP, NBB), dtype=bf16)
    xq8 = np.empty((GP, R), dtype=np.uint8)
    sum8 = np.zeros((G, JB, CPB))
    for g in range(G):
        rows = slice(g * T, (g + 1) * T)
        b0 = g * JB
        q = np.clip(np.round((e_core[b0] - QLO) / QSCALE), 0, 255).astype(np.uint8)
        # [CPB, CLEN, T] -> [T, CLEN, CPB]
        dq8[rows] = q[l_of, :].transpose(2, 1, 0).reshape(T, NB8)
        vb = dqb[rows].reshape(T, CLEN, 3, CPB)
        for j in (1, 2, 3):
            De = np.exp(e_core[b0 + j].astype(np.float32) - KAPPA).astype(bf16)
            vb[:, :, j - 1, :] = De[l_of, :].transpose(2, 1, 0)
        for j in range(JB):
            b = b0 + j
            val = logrho[None, :] + e_core[b, lm1, :] - KAPPA  # [CPB, T]
            qx = np.clip(np.round((val - BXLO) / QSX), 0, 255).astype(np.uint8)
            xq8[rows, XPOS[j] * CPB : (XPOS[j] + 1) * CPB] = qx.T
            x1 = np.exp(QSX * qx.astype(np.float32) + BXLO).astype(bf16)
            sum8[g, j] = x1.astype(np.float64).sum(1)
    boot = np.empty((GP, BOOT_W + R), dtype=np.uint8)
    boot[:, 0:192] = wmat_bytes
    boot[:, 192:196] = np.frombuffer(
        np.full(GP, QLO - KAPPA, np.float32).tobytes(), np.uint8
    ).reshape(GP, 4)
    boot[:, 196:200] = np.frombuffer(
        np.full(GP, BXLO, np.float32).tobytes(), np.uint8
    ).reshape(GP, 4)
    boot[:, BOOT_W:] = xq8
    return {"boot": boot, "dq8": dq8, "dqb": dqb}, sum8


def _chunk0_logsum(e_b, start_f, Ef64):
    """Exact log sum(alpha_{CLEN-1}) for one batch row, float64."""
    a = np.exp(start_f.astype(np.float64) + e_b[0].astype(np.float64))
    for l in range(1, CLEN):
        m = a.max()
        a = ((a / m) @ Ef64) * np.exp(e_b[l].astype(np.float64))
        a *= m
    return np.log(a.sum())


def _assemble_core(xfin, sum8, e_core, start_f, end_f, Ef64):
    """Host combine for one core -> logZ [B_CORE] (float64)."""
    w = np.exp(end_f.astype(np.float64))
    logZ = np.zeros(B_CORE)
    for g in range(G):
        rows = slice(g * T, (g + 1) * T)
        s72 = xfin[rows].astype(np.float64)
        sum72 = s72.sum(0)
        for j in range(JB):
            b = g * JB + j
            cols = slice(j * CPB, (j + 1) * CPB)
            A = np.log(sum72[cols]) + CLEN * KAPPA
            A[1:] -= np.log(sum8[g, j, 1:])
            A0 = _chunk0_logsum(e_core[b], start_f, Ef64)
            xlast = s72[:, j * CPB + (CPB - 1)]
            logZ[b] = A0 + A[1:].sum() + np.log(xlast @ w) - np.log(xlast.sum())
    return logZ


def _host_score(emissions, tags, mask, transitions, start_f, end_f):
    tags = np.asarray(tags).astype(np.int64)
    maskf = np.asarray(mask).astype(np.float64)
    emit = np.take_along_axis(
        emissions, tags[:, :, None], axis=2
    )[..., 0].astype(np.float64)
    score = start_f.astype(np.float64)[tags[:, 0]] + (emit * maskf).sum(1)
    tr = transitions.astype(np.float64)[tags[:, :-1], tags[:, 1:]]
    score += (tr * maskf[:, 1:]).sum(1)
    last_idx = maskf.astype(np.int64).sum(1) - 1
    last_tags = np.take_along_axis(tags, last_idx[:, None], axis=1)[:, 0]
    score += end_f.astype(np.float64)[last_tags]
    return score


def kernel(
    emissions, tags, mask, transitions, start_transitions, end_transitions,
    _trace=False,
):
    from concourse.bass_utils import run_bass_kernel_spmd

    emissions = np.asarray(emissions, dtype=np.float32)
    transitions = np.asarray(transitions, dtype=np.float32)
    start_f = np.asarray(start_transitions, dtype=np.float32)
    end_f = np.asarray(end_transitions, dtype=np.float32)

    E_d = np.exp(transitions).astype(bf16)
    Ef64 = np.exp(transitions.astype(np.float64))
    wmat = _build_wmat(E_d)
    wmat_bytes = wmat.view(np.uint8)  # [GP, 192]
    # rho = E^T @ uniform: the warmup step's matmul output direction
    logrho = np.log(Ef64.sum(0) / T).astype(np.float32)  # [T]

    in_maps = []
    sum8s = []
    for core in range(NCORES):
        e_core = emissions[core * B_CORE : (core + 1) * B_CORE]
        im, s8 = _build_core_inputs(e_core, wmat_bytes, logrho)
        in_maps.append(im)
        sum8s.append(s8)

    nc = _get_nc()
    res = run_bass_kernel_spmd(
        nc, in_maps, core_ids=list(range(NCORES)), trace=_trace
    )
    _CACHE["last_results"] = res

    logZ = np.zeros(B)
    for core in range(NCORES):
        out = res.results[core]
        e_core = emissions[core * B_CORE : (core + 1) * B_CORE]
        logZ[core * B_CORE : (core + 1) * B_CORE] = _assemble_core(
            out["xfin"], sum8s[core], e_core, start_f, end_f, Ef64
        )

    score = _host_score(
        emissions, tags, mask, transitions, start_f, end_f
    )
    return (logZ - score).astype(np.float32)
